# revision 1
# baseline (speedup 1.0000x reference)
"""Trainium2 Bass kernel for a pre-LN causal transformer block.

Sharding: data-parallel over (batch, sequence-half) -> 8 uniform SPMD shards.
Each core handles 1024 queries of one batch against that batch's 2048 keys,
with causality enforced by a host-supplied multiplicative mask applied after
exp (so the SPMD program is identical on every core).

Layout: all activations are E-major ("transposed", [E, tokens]) so every
matmul contraction lands on the partition dim with zero on-device transposes.
LayerNorm is folded into the projections:
    h = (x - mu) * r * gamma + beta;  h @ W
      = r_t * (x @ (gamma*W)) + (-mu_t) * colsum(gamma*W) + beta @ W
so the device computes raw = (gamma*W)^T @ xT, adds the rank-2 correction
( rows (-mu, 1/r) x rows (colsum, beta@W) ) via a K=128-padded matmul into
PSUM, and multiplies by r_t during the PSUM->SBUF copy.

Matmul inputs are bf16 (full PE rate), accumulation f32 in PSUM, softmax and
residuals f32.
"""

import math
from contextlib import ExitStack

import numpy as np
import ml_dtypes

import concourse.bass as bass
import concourse.tile as tile
from concourse import bacc
from concourse.tile import add_dep_helper
from concourse import mybir
from concourse.bass_utils import run_bass_kernel_spmd

F32 = mybir.dt.float32
BF16 = mybir.dt.bfloat16
AF = mybir.ActivationFunctionType

# Full-size problem dims (hardcoded; the harness provides x of this shape).
DIMS = dict(B=4, C=2048, E=1024, H=16, D=64, FF=4096, EPS=1e-5)
N_CORES = 8
P = 128


def _ceil_div(a, b):
    return (a + b - 1) // b


def coalesce_sem_updates(nc):
    """Drop sem increments whose cumulative value no wait references,
    folding their count into the next surviving increment on the same
    engine stream. Sound: every waited-on satisfaction point keeps its
    original cumulative value and firing position; only unobserved
    intermediate values are delayed. Motivated by this environment's
    ~5-8us cost per semaphore op.

    DISABLED: sound per the static satisfaction-point model, but CoreSim
    (and so presumably hardware) deadlocks with folding enabled -- the
    runtime's sem semantics (likely HWDGE queue mapping) differ from the
    abstraction (per-engine stream sems are read implicitly by the runtime
    at points invisible in sync_info -- drains/barriers were ruled out as
    the sole mechanism). Disabled; kept as documentation."""
    return 0
    import bass_rust
    insts = [i for bb in nc.m.functions[0].blocks for i in bb.instructions]
    waited = {}
    for i in insts:
        si = getattr(i, "sync_info", None)
        if si is None:
            continue
        for w in si.on_wait:
            ok = (w.sync_type == "semaphore" and w.wait_reg is None
                  and w.wait_mode == "sem-ge-imm")
            waited.setdefault(w.id, set()).add(w.wait_value if ok else None)
    writers = {}
    poison = set()
    for i in insts:
        si = getattr(i, "sync_info", None)
        if si is None:
            continue
        is_dma = type(i).__name__ == "InstDMACopy"
        ekey = "DMA" if is_dma else str(getattr(i, "engine", None))
        for u in si.on_update:
            if (u.sync_type != "semaphore" or u.update_mode != "sem-inc"
                    or u.update_reg is not None or is_dma):
                poison.add(u.id)
            writers.setdefault(u.id, set()).add(ekey)
    for sid, ws in writers.items():
        if len(ws) > 1:
            poison.add(sid)
    for sid, vals in waited.items():
        if None in vals:
            poison.add(sid)

    # sem range-resets (Drain is_reset_sema / EVENT_SEMAPHORE_RANGE_CLEAR)
    # restart a sem's cumulative count; segment the walk at each one.
    resets = {}  # inst idx -> (first, last) sem id range cleared
    for idx, i in enumerate(insts):
        rs = getattr(i, "reset_range_start", None)
        re_ = getattr(i, "reset_range_stop", None)
        if getattr(i, "is_reset_sema", False) and rs is not None:
            resets[idx] = (rs, re_)
        rf = getattr(i, "range_first", None)
        rl = getattr(i, "range_last", None)
        if rf is not None and rl is not None:
            resets[idx] = (rf, rl + 1)

    # locate each sem's updates in stream order
    upd_sites = {}
    for idx, i in enumerate(insts):
        si = getattr(i, "sync_info", None)
        if si is None:
            continue
        for u in si.on_update:
            if u.id in poison or u.id not in writers:
                continue
            upd_sites.setdefault(u.id, []).append(idx)

    sem_name = {}
    for i in insts:
        si = getattr(i, "sync_info", None)
        if si is None:
            continue
        for u in si.on_update:
            if u.ant_name:
                sem_name[u.id] = u.ant_name

    # Drain / EventSemaphore instructions implicitly reference the current
    # cumulative value of every sem (runtime-computed quiescence checks), so
    # they cut segments for ALL sems.
    global_cuts = [idx for idx, i in enumerate(insts)
                   if type(i).__name__ in ("InstDrain", "InstEventSemaphore")]

    new_vals = {}   # (inst_idx, sem_id) -> new update_value (0 = drop)
    ndrop = 0
    for sid, sites in upd_sites.items():
        cut_at = sorted(set(idx for idx, (a, b) in resets.items()
                            if a <= sid < b) | set(global_cuts))
        # split sites into segments between resets
        segments = []
        seg = []
        ci = 0
        for idx in sites:
            while ci < len(cut_at) and cut_at[ci] < idx:
                if seg:
                    segments.append(seg)
                    seg = []
                ci += 1
            seg.append(idx)
        if seg:
            segments.append(seg)
        wvals = sorted(v for v in waited.get(sid, set()) if v is not None)
        for seg in segments:
            cum = 0
            last_kept_cum = 0
            for pos, idx in enumerate(seg):
                si = insts[idx].sync_info
                uval = next(u.update_value for u in si.on_update
                            if u.id == sid)
                cum += uval
                referenced = any(last_kept_cum < w <= cum for w in wvals)
                if referenced or pos == len(seg) - 1:
                    new_vals[(idx, sid)] = cum - last_kept_cum
                    last_kept_cum = cum
                else:
                    new_vals[(idx, sid)] = 0
                    ndrop += 1

    for idx, i in enumerate(insts):
        si = getattr(i, "sync_info", None)
        if si is None:
            continue
        touched = any((idx, u.id) in new_vals for u in si.on_update)
        if not touched:
            continue
        keep = []
        for u in si.on_update:
            nv = new_vals.get((idx, u.id))
            if nv is None:
                keep.append(u)
            elif nv > 0:
                u.update_value = nv
                keep.append(u)
        i.sync_info = bass_rust.SyncInfo(on_wait=list(si.on_wait),
                                         on_update=keep)
    return ndrop


def build_program(dims):
    """Build the SPMD Bass program. Returns nc."""
    B = dims["B"]
    C = dims["C"]
    E = dims["E"]
    H = dims["H"]
    D = dims["D"]
    FF = dims["FF"]
    EPS = dims["EPS"]

    TKV = C                      # kv tokens per core
    TQ = B * C // N_CORES        # query tokens per core
    ES = E // P                  # E subtiles (contraction)
    FS = FF // P                 # FF subtiles
    HPAIRS = H // 2              # head pairs (Qt/Kt partition packing)
    NQUAD = H // 4               # head quads (V projection batches)
    KT = TKV // P                # key tiles
    QTT = TQ // P                # query token tiles
    QC = _ceil_div(TQ, 512)      # 512-wide query chunks
    QW = TQ // QC                # query chunk width (512 normally)
    KVC = _ceil_div(TKV, 512)    # 512-wide kv chunks
    KVW = TKV // KVC
    assert D == 64 and E == H * D

    nc = bacc.Bacc("TRN2", target_bir_lowering=False, debug=False)

    # ---- DRAM I/O ----
    qt_d = nc.dram_tensor("qt", [E, TQ], BF16, kind="ExternalInput")
    kt_d = nc.dram_tensor("kt", [E, TKV], BF16, kind="ExternalInput")
    va_d = nc.dram_tensor("vaug", [TKV, H, 65], BF16, kind="ExternalInput")
    xqres_d = nc.dram_tensor("xqres", [E, TQ], F32, kind="ExternalInput")
    mask_d = nc.dram_tensor("maskT", [TKV, TQ], BF16, kind="ExternalInput")
    wo_d = nc.dram_tensor("wo", [E, E], BF16, kind="ExternalInput")
    w1_d = nc.dram_tensor("w1", [E, FF], BF16, kind="ExternalInput")
    w2_d = nc.dram_tensor("w2", [FF, E], BF16, kind="ExternalInput")
    # fold tensor for LN2: row0 = colsum(W'), rest 0 (padded to 128 rows)
    w1f_d = nc.dram_tensor("w1fold", [P, FF], BF16, kind="ExternalInput")
    b1f_d = nc.dram_tensor("b1f", [P, FS], F32, kind="ExternalInput")
    b2f_d = nc.dram_tensor("b2f", [P, ES], F32, kind="ExternalInput")
    out_d = nc.dram_tensor("outT", [E, TQ], F32, kind="ExternalOutput")

    qt3 = qt_d.rearrange("(s p) t -> p s t", p=P)
    kt3 = kt_d.rearrange("(m p) t -> p m t", p=P)
    va3 = va_d.rearrange("(t p) h c -> p t h c", p=P)
    xqres3 = xqres_d.rearrange("(s p) t -> p s t", p=P)
    mask3 = mask_d.rearrange("(s p) t -> p s t", p=P)
    out3 = out_d.rearrange("(s p) t -> p s t", p=P)

    with tile.TileContext(nc) as tc, ExitStack() as ctx:
        perm = ctx.enter_context(tc.tile_pool(name="perm", bufs=1))
        tmp = ctx.enter_context(tc.tile_pool(name="tmp", bufs=2))
        wstream = ctx.enter_context(tc.tile_pool(name="wstream", bufs=2))
        ps = ctx.enter_context(tc.tile_pool(name="ps", bufs=2, space="PSUM"))
        pso = ctx.enter_context(tc.tile_pool(name="pso", bufs=2, space="PSUM"))
        dpool = ctx.enter_context(tc.tile_pool(name="dpool", bufs=2,
                                               space="DRAM"))

        def bcast_rows(dst, srcrow, nrows, width):
            """Broadcast a [1, width] sbuf row to [nrows, width] via DRAM."""
            row_d = dpool.tile([1, width], srcrow.dtype, tag="row_d")
            nc.sync.dma_start(row_d, srcrow)
            bsrc = bass.AP(tensor=row_d.tensor, offset=row_d.offset,
                           ap=[[0, nrows]] + row_d.ap[1:])
            nc.gpsimd.dma_start(dst, bsrc)

        ones_bf = perm.tile([P, 1], BF16, tag="ones_bf")
        nc.vector.memset(ones_bf, 1.0)

        # ACT LUT table management: Exp and Gelu live in different hardware
        # tables, and walrus's table-switch rides the first activation using
        # the new table -- which then only supports a single sync wait. Emit
        # zero-dependency dummy activations to carry each switch; order them
        # on the ACT stream with same-engine dep edges (no semaphores).
        scr_in = perm.tile([1, 8], F32, tag="scr_in")
        nc.vector.memset(scr_in, 1.0)
        scr_out = perm.tile([1, 8], F32, tag="scr_out")
        dummy_exp = nc.scalar.activation(scr_out, scr_in, AF.Exp)
        act_exp_insts = []

        # Warm up every DVE / PE opcode on scratch so first-use config
        # loads don't ride real (multi-wait) instructions.
        A = mybir.AluOpType
        nc.vector.tensor_copy(scr_out, scr_in)
        nc.vector.tensor_mul(scr_out, scr_in, scr_in)
        nc.vector.tensor_add(scr_out, scr_in, scr_in)
        nc.vector.tensor_sub(scr_out, scr_in, scr_in)
        nc.vector.tensor_scalar(scr_out, scr_in, 0.5, 0.5, A.mult, A.add)
        nc.vector.tensor_scalar_mul(scr_out, scr_in, 0.5)
        nc.vector.tensor_scalar_add(scr_out, scr_in, 0.5)
        nc.vector.reciprocal(scr_out, scr_in)
        nc.vector.scalar_tensor_tensor(scr_out, scr_in, 0.5, scr_in,
                                       A.add, A.add)
        scr_bf = perm.tile([1, 8], BF16, tag="scr_bf")
        nc.vector.memset(scr_bf, 1.0)
        nc.vector.tensor_mul(scr_bf, scr_bf, scr_bf)
        scr_ps = ps.tile([P, 1024], F32, tag="ps")
        nc.tensor.matmul(scr_ps[0:8, 0:8], scr_bf[0:1, 0:8],
                         scr_bf[0:1, 0:8], start=True, stop=True)
        nc.vector.tensor_copy(scr_out, scr_ps[0:1, 0:8])
        b1f_sb = perm.tile([P, FS], F32, tag="b1f")
        nc.sync.dma_start(b1f_sb, b1f_d[:, :])
        b2f_sb = perm.tile([P, ES], F32, tag="b2f")
        nc.sync.dma_start(b2f_sb, b2f_d[:, :])
        hidden = perm.tile([P, HPAIRS, TQ], BF16, tag="hidden")

        # ---------- LN statistics (per token, over E) ----------
        # foldrow[0] = -mu, foldrow[1] = 1/r = sqrt(var+eps); rows 2.. = 0.
        # a_bcast = r broadcast to all 128 partitions (bf16).
        def ln_stats(src_sb, ntok, foldrow, a_bcast, a_colT=None):
            nchunk = _ceil_div(ntok, 512)
            w = ntok // nchunk
            for c in range(nchunk):
                sl = slice(c * w, (c + 1) * w)
                pst = ps.tile([P, 1024], F32, tag="ps")
                psum_s = pst[0:1, 0:w]
                psum_q = pst[0:1, 512:512 + w]
                for s in range(ES):
                    nc.tensor.matmul(psum_s, ones_bf, src_sb[:, s, sl],
                                     start=(s == 0), stop=(s == ES - 1))
                for s in range(ES):
                    sq_s = tmp.tile([P, w], BF16, tag="sq_s")
                    nc.vector.tensor_mul(sq_s, src_sb[:, s, sl],
                                         src_sb[:, s, sl])
                    nc.tensor.matmul(psum_q, ones_bf, sq_s,
                                     start=(s == 0), stop=(s == ES - 1))
                mu = tmp.tile([1, w], F32, tag="mu")
                nc.vector.tensor_scalar_mul(mu, psum_s, 1.0 / E)
                m2 = tmp.tile([1, w], F32, tag="m2")
                nc.vector.tensor_scalar_mul(m2, psum_q, 1.0 / E)
                var = tmp.tile([1, w], F32, tag="var")
                nc.vector.tensor_mul(var, mu, mu)
                nc.vector.tensor_sub(var, m2, var)
                nc.vector.tensor_scalar_add(var, var, EPS)
                # r = rsqrt(var) via reciprocal seed + 3 Newton steps (DVE
                # only -- avoids the ACT Sqrt table). var ~ 1 for LN inputs.
                w_ = tmp.tile([1, w], F32, tag="wrec")
                nc.vector.reciprocal(w_, var)
                r_ = tmp.tile([1, w], F32, tag="rr")
                nc.vector.tensor_scalar(r_, w_, 0.5, 0.5,
                                        mybir.AluOpType.mult,
                                        mybir.AluOpType.add)
                t_ = tmp.tile([1, w], F32, tag="tt")
                for _ in range(3):
                    nc.vector.tensor_mul(t_, r_, r_)
                    nc.vector.tensor_mul(t_, t_, var)
                    nc.vector.tensor_scalar(t_, t_, -0.5, 1.5,
                                            mybir.AluOpType.mult,
                                            mybir.AluOpType.add)
                    nc.vector.tensor_mul(r_, r_, t_)
                irow = tmp.tile([1, w], F32, tag="irow")
                nc.vector.tensor_mul(irow, var, r_)
                # a_bcast row 0 (cast to bf16), then broadcast to rows 1..127
                nc.vector.tensor_copy(a_bcast[0:1, sl], r_)
                # foldrow row0 = -mu (partition 0 -> 0, direct DVE)
                nc.vector.tensor_scalar_mul(foldrow[0:1, sl], mu, -1.0)
                # foldrow row1 = irow (partition 0 -> 1 via DMA)
                nc.gpsimd.dma_start(foldrow[1:2, sl], irow)
            bcast_rows(a_bcast[1:P, :], a_bcast[0:1, :], P - 1, ntok)
            if a_colT is not None:
                row_d = dpool.tile([1, ntok], BF16, tag="row_d")
                nc.sync.dma_start(row_d, a_bcast[0:1, :])
                nc.gpsimd.dma_start(
                    a_colT, row_d[0].rearrange("(t p) -> p t", p=P))

        with tc.tile_pool(name="att", bufs=1) as att:
            mask_sb = att.tile([P, KT, TQ], BF16, tag="mask")
            for kt in range(KT):
                nc.sync.dma_start(mask_sb[:, kt], mask3[:, kt])
            qt_all = att.tile([P, HPAIRS, TQ], BF16, tag="qt_all")
            for m in range(HPAIRS):
                nc.sync.dma_start(qt_all[:, m], qt3[:, m])

            # ---------- per-quad: K/V loads + attention ----------
            with tc.tile_pool(name="quad", bufs=2) as quad, \
                 tc.tile_pool(name="ppool", bufs=4) as ppool:
                for q4 in range(NQUAD):
                    kts = []
                    for pr2 in range(2):
                        m = q4 * 2 + pr2
                        ktp = quad.tile([P, TKV], BF16, tag="ktp")
                        nc.sync.dma_start(ktp, kt3[:, m])
                        kts.append(ktp)

                    vq = quad.tile([P, KT, 4 * 65], BF16, tag="vq")
                    vq_v = vq.rearrange("p t (h c) -> p t h c", c=65)
                    for tt in range(KT):
                        nc.sync.dma_start(
                            vq_v[:, tt],
                            va3[:, tt, q4 * 4:(q4 + 1) * 4, :])

                    # attention for the quad's two pairs
                    for pr2 in range(2):
                        m = q4 * 2 + pr2
                        ktp = kts[pr2]
                        opsA = pso.tile([65, 1024], F32, tag="opsum")
                        opsB = pso.tile([65, 1024], F32, tag="opsum")
                        for kt in range(KT):
                            ksl = slice(kt * P, (kt + 1) * P)
                            for hh, ops in ((0, opsA), (1, opsB)):
                                rows = slice(hh * 64, hh * 64 + 64)
                                sc = ps.tile([P, 1024], F32, tag="ps")
                                for c in range(QC):
                                    qsl = slice(c * QW, (c + 1) * QW)
                                    nc.tensor.matmul(
                                        sc[:, c * 512:c * 512 + QW],
                                        ktp[rows, ksl],
                                        qt_all[rows, m, qsl],
                                        start=True, stop=True)
                                pt = ppool.tile([P, 1024], BF16, tag="pT")
                                _ei = nc.scalar.activation(
                                    pt[:, 0:TQ], sc[:, 0:TQ], AF.Exp)
                                act_exp_insts.append(_ei)
                                add_dep_helper(
                                    _ei.ins, dummy_exp.ins, sync=True,
                                    reason="act table: exp after switch")
                                nc.vector.tensor_tensor(
                                    pt[:, 0:TQ], pt[:, 0:TQ], mask_sb[:, kt],
                                    mybir.AluOpType.mult)
                                h4 = 2 * pr2 + hh
                                vcols = slice(h4 * 65, h4 * 65 + 65)
                                for c in range(QC):
                                    nc.tensor.matmul(
                                        ops[:, c * 512:c * 512 + QW],
                                        vq[:, kt, vcols],
                                        pt[:, c * 512:c * 512 + QW],
                                        start=(kt == 0), stop=(kt == KT - 1))
                        # normalize: hidden = O / sum (sum at psum row 64)
                        for hh, ops in ((0, opsA), (1, opsB)):
                            ssb = tmp.tile([65, TQ], F32, tag="ssb")
                            nc.vector.reciprocal(ssb[64:65], ops[64:65, 0:TQ])
                            rb = tmp.tile([64, TQ], F32, tag="t4")
                            bcast_rows(rb, ssb[64:65, :], 64, TQ)
                            if hh == 0:
                                nc.vector.tensor_tensor(
                                    hidden[0:64, m], ops[0:64, 0:TQ], rb,
                                    mybir.AluOpType.mult)
                            else:
                                hb = tmp.tile([64, TQ], BF16, tag="hb")
                                nc.vector.tensor_tensor(
                                    hb, ops[0:64, 0:TQ], rb,
                                    mybir.AluOpType.mult)
                                nc.gpsimd.dma_start(hidden[64:128, m], hb)

        # ---------- Wo + residual, LN2, FFN ----------
        with tc.tile_pool(name="post", bufs=1) as post:
            out1 = post.tile([P, ES, TQ], F32, tag="out1")
            out1bf = post.tile([P, ES, TQ], BF16, tag="out1bf")
            wo3 = wo_d.rearrange("(s p) e -> p s e", p=P)
            for et in range(ES):
                wo_et = wstream.tile([P, ES, P], BF16, tag="w")
                nc.sync.dma_start(wo_et, wo3[:, :, et * P:(et + 1) * P])
                pst = ps.tile([P, 1024], F32, tag="ps")
                for c in range(QC):
                    psl = pst[:, c * 512:c * 512 + QW]
                    qsl = slice(c * QW, (c + 1) * QW)
                    for s in range(ES):
                        nc.tensor.matmul(psl, wo_et[:, s], hidden[:, s, qsl],
                                         start=(s == 0), stop=(s == ES - 1))
                xr = tmp.tile([P, TQ], F32, tag="t4")
                nc.sync.dma_start(xr, xqres3[:, et])
                nc.vector.tensor_add(out1[:, et], pst[:, 0:TQ], xr)
                nc.vector.tensor_copy(out1bf[:, et], out1[:, et])

            foldrow2 = post.tile([P, TQ], BF16, tag="foldrow2")
            nc.vector.memset(foldrow2, 0.0)
            a2 = post.tile([P, TQ], BF16, tag="a2")
            ln_stats(out1bf, TQ, foldrow2, a2)

            scr_out2 = perm.tile([1, 8], F32, tag="scr_out2")
            dummy_gelu = nc.scalar.activation(scr_out2, scr_in, AF.Gelu)
            for ei in act_exp_insts:
                add_dep_helper(dummy_gelu.ins, ei.ins, sync=True,
                               reason="act table: gelu after all exps")

            h3 = post.tile([P, FS, TQ], BF16, tag="h3")
            w1f_sb = post.tile([P, FF], BF16, tag="w1f")
            nc.sync.dma_start(w1f_sb, w1f_d[:, :])
            w13 = w1_d.rearrange("(s p) f -> p s f", p=P)
            for ft in range(FS):
                w1_ft = wstream.tile([P, ES, P], BF16, tag="w")
                nc.sync.dma_start(w1_ft, w13[:, :, ft * P:(ft + 1) * P])
                pst = ps.tile([P, 1024], F32, tag="ps")
                for c in range(QC):
                    psl = pst[:, c * 512:c * 512 + QW]
                    qsl = slice(c * QW, (c + 1) * QW)
                    for s in range(ES):
                        nc.tensor.matmul(psl, w1_ft[:, s], out1bf[:, s, qsl],
                                         start=(s == 0), stop=False)
                    nc.tensor.matmul(psl, w1f_sb[:, ft * P:(ft + 1) * P],
                                     foldrow2[:, qsl], start=False, stop=True)
                mid = tmp.tile([P, TQ], F32, tag="t4")
                nc.vector.tensor_tensor(mid, pst[:, 0:TQ], a2,
                                        mybir.AluOpType.mult)
                gi = nc.scalar.activation(h3[:, ft], mid, AF.Gelu,
                                          bias=b1f_sb[:, ft:ft + 1])
                add_dep_helper(gi.ins, dummy_gelu.ins, sync=True,
                               reason="act table: gelu after switch")

            w23 = w2_d.rearrange("(s p) e -> p s e", p=P)
            for et in range(ES):
                w2_et = wstream.tile([P, FS, P], BF16, tag="w")
                nc.sync.dma_start(w2_et, w23[:, :, et * P:(et + 1) * P])
                pst = ps.tile([P, 1024], F32, tag="ps")
                for c in range(QC):
                    psl = pst[:, c * 512:c * 512 + QW]
                    qsl = slice(c * QW, (c + 1) * QW)
                    for s in range(FS):
                        nc.tensor.matmul(psl, w2_et[:, s], h3[:, s, qsl],
                                         start=(s == 0), stop=(s == FS - 1))
                ot = tmp.tile([P, TQ], F32, tag="t4")
                nc.vector.scalar_tensor_tensor(
                    ot, pst[:, 0:TQ], b2f_sb[:, et:et + 1], out1[:, et],
                    mybir.AluOpType.add, mybir.AluOpType.add)
                nc.sync.dma_start(out3[:, et], ot)

    nc.compile()
    n = coalesce_sem_updates(nc)
    return nc


# ---------------------------------------------------------------------------
# Host side
# ---------------------------------------------------------------------------

def prep_inputs(dims, x, ln1_g, ln1_b, Wq, Wk, Wv, Wo, ln2_g, ln2_b,
                W1, b1, W2, b2):
    """Build per-core in_maps (list of dicts keyed by dram tensor names)."""
    B, C, E, H, D, FF = (dims["B"], dims["C"], dims["E"], dims["H"],
                         dims["D"], dims["FF"])
    TQ = B * C // N_CORES
    bf = ml_dtypes.bfloat16
    f32 = np.float32

    x = np.asarray(x, f32)
    sc = 1.0 / math.sqrt(D)
    wq = np.asarray(Wq, f32) * sc
    wk = np.asarray(Wk, f32)
    wv = np.asarray(Wv, f32)
    w1 = ln2_g[:, None] * np.asarray(W1, f32)
    b1f = np.asarray(b1, f32) + ln2_b @ np.asarray(W1, f32)
    # LN1 is a pure function of the input: apply it on the host.
    mu = x.mean(-1, keepdims=True)
    var = x.var(-1, keepdims=True)
    h1 = ((x - mu) / np.sqrt(var + 1e-5)) * np.asarray(ln1_g, f32) \
        + np.asarray(ln1_b, f32)

    def fold(w, bias):
        f = np.zeros((P, w.shape[1]), f32)
        f[0] = w.sum(axis=0)
        f[1] = bias
        return f.astype(bf)

    shared = {
        "wo": np.asarray(Wo, f32).astype(bf),
        "w1": w1.astype(bf), "w2": np.asarray(W2, f32).astype(bf),
        "w1fold": fold(w1, np.zeros(FF, f32)),
        "b1f": np.ascontiguousarray(b1f.reshape(FF // P, P).T),
        "b2f": np.ascontiguousarray(np.asarray(b2, f32).reshape(E // P, P).T),
    }

    nhalf = C // TQ  # query shards per batch
    in_maps = []
    for c in range(N_CORES):
        b = c // nhalf
        off = (c % nhalf) * TQ
        xqf = x[b, off:off + TQ]               # [TQ, E]
        kpos = np.arange(C)[:, None]
        qpos = np.arange(TQ)[None, :] + off
        # Q/K/V are pure functions of the inputs: project on the host.
        Qh = h1[b, off:off + TQ] @ wq          # [TQ, E]
        Kh = h1[b] @ wk                        # [TKV, E]
        Vh = h1[b] @ wv                        # [TKV, E]
        va = np.ones((C, len(Vh[0]) // 64, 65), np.float32)
        va[:, :, :64] = Vh.reshape(C, -1, 64)
        m = {
            "qt": np.ascontiguousarray(Qh.T).astype(bf),
            "kt": np.ascontiguousarray(Kh.T).astype(bf),
            "vaug": va.astype(bf),
            "xqres": np.ascontiguousarray(xqf.T),
            "maskT": (kpos <= qpos).astype(bf),
        }
        m.update(shared)
        in_maps.append(m)
    return in_maps


def assemble_output(dims, results):
    B, C, E = dims["B"], dims["C"], dims["E"]
    TQ = B * C // N_CORES
    nhalf = C // TQ
    out = np.empty((B, C, E), np.float32)
    for c in range(N_CORES):
        b = c // nhalf
        off = (c % nhalf) * TQ
        out[b, off:off + TQ] = results[c]["outT"].T
    return out


def kernel(**inputs):
    dims = DIMS
    nc = build_program(dims)
    in_maps = prep_inputs(dims, **{k: np.asarray(v) for k, v in
                                   inputs.items()})
    res = run_bass_kernel_spmd(nc, in_maps, list(range(N_CORES)))
    return assemble_output(dims, res.results)


if __name__ == "__main__":
    nc = build_program(DIMS)
    print("build ok")



# revision 10
# speedup vs baseline: 2.8493x; 2.8493x over previous
"""Trainium2 Bass kernel for a pre-LN causal transformer block.

The dispatch cost in this environment is dominated by per-dispatch input
staging (~0.65 ms per MB of per-core input), so the design minimizes
per-core I/O bytes:

  - Weights are sharded 1/8 per core and AllGather'd on device (blobA:
    [W'q|W'k|W'v|Wo|W2] rows, blobB: W1), so each core ships 3MB of
    weights instead of 24MB.
  - Each core ships only its own 1024 tokens of x (bf16, E-major); the
    batch pair exchanges halves with a 2-core AllGather so K/V cover all
    2048 keys.
  - Q/K/V projections, the V transpose (PE identity matmul), and the
    causal mask (is_ge against shipped q/k index rows) are computed on
    device.
  - LayerNorms are folded into the projections:
        h = (x - mu) * r * gamma + beta;  h @ W
          = r_t * (x @ (gamma*W)) + (-mu_t)*colsum(gamma*W)*r_t + beta@W
    realized as a rank-2 correction matmul into the same PSUM
    accumulation, with the r_t multiply fused into the PSUM->SBUF copy.
    LN statistics for LN1 come from the host (tiny rows); LN2 stats are
    computed on device.
  - Output is bf16 [E, TQ].

Matmul inputs are bf16 (full PE rate), accumulation f32 in PSUM, softmax
and residual f32.
"""

import math
import os
from contextlib import ExitStack

import numpy as np
import ml_dtypes

import concourse.bass as bass
import concourse.tile as tile
from concourse import bacc
from concourse.tile import add_dep_helper
from concourse import mybir
from concourse.bass_utils import run_bass_kernel_spmd

F32 = mybir.dt.float32
BF16 = mybir.dt.bfloat16
FP8 = mybir.dt.float8e4
AF = mybir.ActivationFunctionType
A = mybir.AluOpType

# Full-size problem dims (hardcoded; the harness provides x of this shape).
DIMS = dict(B=4, C=2048, E=1024, H=16, D=64, FF=4096, EPS=1e-5)
N_CORES = 8
P = 128


def build_program(dims):
    """Build the SPMD Bass program. Returns nc."""
    B = dims["B"]
    C = dims["C"]
    E = dims["E"]
    H = dims["H"]
    D = dims["D"]
    FF = dims["FF"]
    EPS = dims["EPS"]

    TKV = C                      # kv tokens per core (full batch)
    TQ = B * C // N_CORES        # query tokens per core (1024)
    ES = E // P                  # E subtiles (contraction) = 8
    FS = FF // P                 # FF subtiles = 32
    HPAIRS = H // 2              # head pairs = 8
    NQUAD = H // 4               # head quads = 4
    KT = TKV // P                # key tiles = 16
    QC = TQ // 512               # 512-wide query chunks = 2
    NG = TKV // 1024             # 1024-token groups of kv = 2
    assert D == 64 and E == H * D

    # blobA row offsets
    AQ, AK, AV, AO, A2 = 0, E, 2 * E, 3 * E, 4 * E
    AROWS = 4 * E + FF           # 8192
    ASH = AROWS // N_CORES       # 1024 rows per core
    BSH = E // N_CORES           # 128 rows per core of blobB

    nc = bacc.Bacc("TRN2", target_bir_lowering=False, debug=False,
                   num_devices=N_CORES)

    # ---- DRAM I/O (per-core) ----
    xq_d = nc.dram_tensor("xq", [E, TQ], BF16, kind="ExternalInput")
    st_d = nc.dram_tensor("stats", [4, TQ], F32, kind="ExternalInput")
    wa_d = nc.dram_tensor("wa", [ASH, E], BF16, kind="ExternalInput")
    wb_d = nc.dram_tensor("wb", [BSH, FF], BF16, kind="ExternalInput")
    fo_d = nc.dram_tensor("folds", [4, FF], F32, kind="ExternalInput")
    qo_d = nc.dram_tensor("qoff", [1, TQ], F32, kind="ExternalInput")
    ki_d = nc.dram_tensor("kidx", [P, KT], F32, kind="ExternalInput")
    id_d = nc.dram_tensor("ident", [P, P], BF16, kind="ExternalInput")
    b1f_d = nc.dram_tensor("b1f", [P, FS], F32, kind="ExternalInput")
    b2f_d = nc.dram_tensor("b2f", [P, ES], F32, kind="ExternalInput")
    out_d = nc.dram_tensor("outT", [E, TQ], BF16, kind="ExternalOutput")
    KDBG = bool(os.environ.get("KDBG"))
    if KDBG:
        dbg_wag = nc.dram_tensor("dbg_wag", [AROWS, E], BF16,
                                 kind="ExternalOutput")
        dbg_xg = nc.dram_tensor("dbg_xg", [2, E, TQ], BF16,
                                kind="ExternalOutput")
        dbg_sg = nc.dram_tensor("dbg_sg", [2, 4, TQ], F32,
                                kind="ExternalOutput")
        dbg_mask = nc.dram_tensor("dbg_mask", [P, KT, TQ], FP8,
                                  kind="ExternalOutput")
        dbg_qt = nc.dram_tensor("dbg_qt", [P, HPAIRS, TQ], BF16,
                                kind="ExternalOutput")
        dbg_kt = nc.dram_tensor("dbg_kt", [P, TKV], BF16,
                                kind="ExternalOutput")
        dbg_vq = nc.dram_tensor("dbg_vq", [P, KT, 4 * 65], BF16,
                                kind="ExternalOutput")
        dbg_hid = nc.dram_tensor("dbg_hid", [P, HPAIRS, TQ], BF16,
                                 kind="ExternalOutput")

    xq3 = xq_d.rearrange("(s p) t -> p s t", p=P)
    out3 = out_d.rearrange("(s p) t -> p s t", p=P)
    PAIRS = [[2 * i, 2 * i + 1] for i in range(N_CORES // 2)]
    FULLG = [list(range(N_CORES))]

    with tile.TileContext(nc) as tc, ExitStack() as ctx:
        perm = ctx.enter_context(tc.tile_pool(name="perm", bufs=1))
        tmp = ctx.enter_context(tc.tile_pool(name="tmp", bufs=2))
        wstream = ctx.enter_context(tc.tile_pool(name="wstream", bufs=2))
        ps = ctx.enter_context(tc.tile_pool(name="ps", bufs=2, space="PSUM"))
        pso = ctx.enter_context(tc.tile_pool(name="pso", bufs=2, space="PSUM"))
        dpool = ctx.enter_context(tc.tile_pool(name="dpool", bufs=2,
                                               space="DRAM"))
        dgath = ctx.enter_context(tc.tile_pool(name="dgath", bufs=1,
                                               space="DRAM"))

        # ---------- collectives: gather x halves, stats, weights ----------
        xb = dgath.tile([E, TQ], BF16, tag="xb")
        xg = dgath.tile([2, E, TQ], BF16, tag="xg")
        sb = dgath.tile([4, TQ], F32, tag="sb")
        sg = dgath.tile([2, 4, TQ], F32, tag="sg")
        wab = dgath.tile([ASH, E], BF16, tag="wab")
        wag = dgath.tile([AROWS, E], BF16, tag="wag")
        wbb = dgath.tile([BSH, FF], BF16, tag="wbb")
        wbg = dgath.tile([E, FF], BF16, tag="wbg")

        nc.gpsimd.dma_start(xb[:], xq_d[:, :])
        nc.gpsimd.dma_start(sb[:], st_d[:, :])
        nc.gpsimd.dma_start(wab[:], wa_d[:, :])
        nc.gpsimd.dma_start(wbb[:], wb_d[:, :])
        nc.gpsimd.collective_compute(
            "AllGather", A.bypass, replica_groups=PAIRS,
            ins=[xb.opt()], outs=[xg.opt()])
        nc.gpsimd.collective_compute(
            "AllGather", A.bypass, replica_groups=PAIRS,
            ins=[sb.opt()], outs=[sg.opt()])
        # qkv rows first so projections can start before wo/w2 arrive
        qkv_sh = 3 * E // N_CORES   # 384 per-core rows of the qkv part
        nc.gpsimd.collective_compute(
            "AllGather", A.bypass, replica_groups=FULLG,
            ins=[wab[0:qkv_sh, :].opt()], outs=[wag[0:3 * E, :].opt()])
        nc.gpsimd.collective_compute(
            "AllGather", A.bypass, replica_groups=FULLG,
            ins=[wab[qkv_sh:ASH, :].opt()],
            outs=[wag[3 * E:AROWS, :].opt()])
        nc.gpsimd.collective_compute(
            "AllGather", A.bypass, replica_groups=FULLG,
            ins=[wbb.opt()], outs=[wbg.opt()])

        if KDBG:
            nc.gpsimd.dma_start(dbg_wag[:, :], wag[:])
            nc.gpsimd.dma_start(dbg_xg[:, :, :], xg[:])
            nc.gpsimd.dma_start(dbg_sg[:, :, :], sg[:])

        # weight views into the gathered blobs
        wq3 = wag[AQ:AQ + E, :].rearrange("(s p) n -> p s n", p=P)
        wk3 = wag[AK:AK + E, :].rearrange("(s p) n -> p s n", p=P)
        wv3 = wag[AV:AV + E, :].rearrange("(s p) n -> p s n", p=P)
        wo3 = wag[AO:AO + E, :].rearrange("(s p) e -> p s e", p=P)
        w23 = wag[A2:A2 + FF, :].rearrange("(s p) e -> p s e", p=P)
        w13 = wbg.rearrange("(s p) f -> p s f", p=P)

        def bcast_rows(dst, srcrow, nrows, width):
            """Broadcast a [1, width] sbuf row to [nrows, width] via DRAM."""
            row_d = dpool.tile([1, width], srcrow.dtype, tag="row_d")
            nc.sync.dma_start(row_d, srcrow)
            bsrc = bass.AP(tensor=row_d.tensor, offset=row_d.offset,
                           ap=[[0, nrows]] + row_d.ap[1:])
            nc.gpsimd.dma_start(dst, bsrc)

        def bcast_dram_row(dst, src_ap, nrows, width, dtype):
            """Broadcast a [1, width] DRAM row to [nrows, width] sbuf."""
            row_d = dpool.tile([1, width], dtype, tag="row_d")
            nc.sync.dma_start(row_d, src_ap)
            bsrc = bass.AP(tensor=row_d.tensor, offset=row_d.offset,
                           ap=[[0, nrows]] + row_d.ap[1:])
            nc.gpsimd.dma_start(dst, bsrc)

        ones_bf = perm.tile([P, 1], BF16, tag="ones_bf")
        nc.vector.memset(ones_bf, 1.0)

        # ACT LUT table management: Exp and Gelu live in different hardware
        # tables; emit zero-dependency dummy activations to carry each
        # switch, ordered on the ACT stream with same-engine dep edges.
        scr_in = perm.tile([1, 8], F32, tag="scr_in")
        nc.vector.memset(scr_in, 1.0)
        scr_out = perm.tile([1, 8], F32, tag="scr_out")
        dummy_exp = nc.scalar.activation(scr_out, scr_in, AF.Exp)
        act_exp_insts = []

        # Warm up engine opcodes on scratch so first-use config loads
        # don't ride real (multi-wait) instructions.
        nc.vector.tensor_copy(scr_out, scr_in)
        nc.vector.tensor_mul(scr_out, scr_in, scr_in)
        nc.vector.tensor_add(scr_out, scr_in, scr_in)
        nc.vector.tensor_sub(scr_out, scr_in, scr_in)
        nc.vector.tensor_scalar(scr_out, scr_in, 0.5, 0.5, A.mult, A.add)
        nc.vector.tensor_scalar(scr_out, scr_in, 0.5, None, A.is_ge)
        nc.vector.tensor_scalar_mul(scr_out, scr_in, 0.5)
        nc.vector.tensor_scalar_add(scr_out, scr_in, 0.5)
        nc.vector.reciprocal(scr_out, scr_in)
        nc.vector.scalar_tensor_tensor(scr_out, scr_in, 0.5, scr_in,
                                       A.add, A.add)
        scr_bf = perm.tile([1, 8], BF16, tag="scr_bf")
        nc.vector.memset(scr_bf, 1.0)
        nc.vector.tensor_mul(scr_bf, scr_bf, scr_bf)
        scr_ps = ps.tile([P, 1024], F32, tag="ps")
        nc.tensor.matmul(scr_ps[0:8, 0:8], scr_bf[0:1, 0:8],
                         scr_bf[0:1, 0:8], start=True, stop=True)
        nc.vector.tensor_copy(scr_out, scr_ps[0:1, 0:8])

        b1f_sb = perm.tile([P, FS], F32, tag="b1f")
        nc.sync.dma_start(b1f_sb, b1f_d[:, :])
        b2f_sb = perm.tile([P, ES], F32, tag="b2f")
        nc.sync.dma_start(b2f_sb, b2f_d[:, :])
        hidden = perm.tile([P, HPAIRS, TQ], BF16, tag="hidden")

        with tc.tile_pool(name="att", bufs=1) as att:
            ident_sb = att.tile([P, P], BF16, tag="ident")
            nc.sync.dma_start(ident_sb, id_d[:, :])
            kidx_sb = att.tile([P, KT], F32, tag="kidx")
            nc.sync.dma_start(kidx_sb, ki_d[:, :])
            # LN1 fold rows (host stats): row0=-mu, row1=1/r, rows 2..=0
            foldrowQ = att.tile([P, TQ], BF16, tag="frQ")
            nc.vector.memset(foldrowQ, 0.0)
            nc.gpsimd.dma_start(foldrowQ[0:1, :], st_d[0:1, :])
            nc.gpsimd.dma_start(foldrowQ[1:2, :], st_d[1:2, :])
            foldrowK = att.tile([P, TKV], BF16, tag="frK")
            nc.vector.memset(foldrowK, 0.0)
            for h in range(2):
                hsl = slice(h * TQ, (h + 1) * TQ)
                nc.gpsimd.dma_start(foldrowK[0:1, hsl], sg[h, 0:1, :])
                nc.gpsimd.dma_start(foldrowK[1:2, hsl], sg[h, 1:2, :])
            # r broadcast tiles (f32)
            rbcQ = att.tile([P, TQ], F32, tag="rbcQ")
            bcast_dram_row(rbcQ, st_d[2:3, :], P, TQ, F32)
            rbcK = att.tile([P, TKV], F32, tag="rbcK")
            for h in range(2):
                hsl = slice(h * TQ, (h + 1) * TQ)
                bcast_dram_row(rbcK[:, hsl], sg[h, 2:3, :], P, TQ, F32)
            # q-position broadcast (f32) for the causal mask
            qb = att.tile([P, TQ], F32, tag="qb")
            bcast_dram_row(qb, qo_d[0:1, :], P, TQ, F32)
            # fold stationary for q/k/v: row0 = colsum(W'), row1 = beta@W'
            foldWqkv = att.tile([P, 3 * E], BF16, tag="foldWqkv")
            nc.vector.memset(foldWqkv, 0.0)
            nc.gpsimd.dma_start(foldWqkv[0:1, :], fo_d[0:1, 0:3 * E])
            nc.gpsimd.dma_start(foldWqkv[1:2, :], fo_d[1:2, 0:3 * E])
            # causal mask: mask[p, kt, q] = (qoff[q] >= kidx[p, kt])
            mask_sb = att.tile([P, KT, TQ], FP8, tag="mask")
            for j in range(KT):
                nc.vector.tensor_scalar(mask_sb[:, j], qb,
                                        kidx_sb[:, j:j + 1], None, A.is_ge)

            # gathered x (E-major, both halves of the batch)
            if KDBG:
                nc.gpsimd.dma_start(dbg_mask[:, :, :], mask_sb)

            xsb = att.tile([P, ES, TKV], BF16, tag="xsb")
            for h in range(2):
                nc.sync.dma_start(
                    xsb[:, :, h * TQ:(h + 1) * TQ],
                    xg[h].rearrange("(s p) t -> p s t", p=P))

            def project(dst_ap, wview, fold_off, moving, frow, rbc, ntok,
                        m):
                """One 128-col block of a folded-LN projection.

                dst_ap: [P, ntok] bf16 destination (r-mult applied)
                wview:  [p, s, n] DRAM view of W' (E-contraction)
                fold_off: column offset of this projection in foldWqkv
                moving: [P, ES, ntok] bf16 sbuf
                """
                wt = wstream.tile([P, ES, P], BF16, tag="w")
                nc.sync.dma_start(wt, wview[:, :, m * P:(m + 1) * P])
                for g in range(ntok // 1024):
                    pst = ps.tile([P, 1024], F32, tag="ps")
                    for c in range(2):
                        tsl = slice(g * 1024 + c * 512,
                                    g * 1024 + (c + 1) * 512)
                        psl = pst[:, c * 512:(c + 1) * 512]
                        for s in range(ES):
                            nc.tensor.matmul(psl, wt[:, s], moving[:, s, tsl],
                                             start=(s == 0), stop=False)
                        nc.tensor.matmul(
                            psl,
                            foldWqkv[:, fold_off + m * P:fold_off + (m + 1) * P],
                            frow[:, tsl], start=False, stop=True)
                    gsl = slice(g * 1024, (g + 1) * 1024)
                    nc.vector.tensor_tensor(dst_ap[:, gsl], pst[:, 0:1024],
                                            rbc[:, gsl], A.mult)

            # Q for own tokens (all head pairs)
            qt_all = att.tile([P, HPAIRS, TQ], BF16, tag="qt_all")
            with tc.tile_pool(name="proj", bufs=1) as proj:
                xqsb = proj.tile([P, ES, TQ], BF16, tag="xqsb")
                nc.sync.dma_start(xqsb, xq3[:, :, :])
                for m in range(HPAIRS):
                    project(qt_all[:, m], wq3, 0, xqsb, foldrowQ, rbcQ,
                            TQ, m)

            if KDBG:
                nc.gpsimd.dma_start(dbg_qt[:, :, :], qt_all)

            # K/V per quad + attention
            if True:
                with tc.tile_pool(name="quad", bufs=2) as quad, \
                     tc.tile_pool(name="ppool", bufs=4) as ppool:
                    for q4 in range(NQUAD):
                        vq = quad.tile([P, KT, 4 * 65], BF16, tag="vq")
                        nc.vector.memset(vq, 1.0)
                        vq_v = vq.rearrange("p t (h c) -> p t h c", c=65)
                        kts = []
                        for pr2 in range(2):
                            m = q4 * 2 + pr2
                            ktp = quad.tile([P, TKV], BF16, tag="ktp")
                            project(ktp, wk3, E, xsb, foldrowK, rbcK,
                                    TKV, m)
                            kts.append(ktp)
                            # V block + transpose into vq
                            wtv = wstream.tile([P, ES, P], BF16, tag="w")
                            nc.sync.dma_start(wtv,
                                              wv3[:, :, m * P:(m + 1) * P])
                            for g in range(NG):
                                pst = ps.tile([P, 1024], F32, tag="ps")
                                for c in range(2):
                                    tsl = slice(g * 1024 + c * 512,
                                                g * 1024 + (c + 1) * 512)
                                    psl = pst[:, c * 512:(c + 1) * 512]
                                    for s in range(ES):
                                        nc.tensor.matmul(
                                            psl, wtv[:, s], xsb[:, s, tsl],
                                            start=(s == 0), stop=False)
                                    nc.tensor.matmul(
                                        psl,
                                        foldWqkv[:, 2 * E + m * P:
                                                 2 * E + (m + 1) * P],
                                        foldrowK[:, tsl],
                                        start=False, stop=True)
                                vtmp = tmp.tile([P, 1024], BF16, tag="vtmp")
                                gsl = slice(g * 1024, (g + 1) * 1024)
                                nc.vector.tensor_tensor(
                                    vtmp, pst[:, 0:1024], rbcK[:, gsl],
                                    A.mult)
                                pst2 = ps.tile([P, 1024], F32, tag="ps")
                                for j in range(8):
                                    kt = g * 8 + j
                                    jsl = slice(j * P, (j + 1) * P)
                                    # transpose: out[i,j] = sum_p v[p,i]*I[p,j]
                                    nc.tensor.matmul(
                                        pst2[:, jsl], vtmp[:, jsl], ident_sb,
                                        start=True, stop=True)
                                    nc.vector.tensor_copy(
                                        vq_v[:, kt, 2 * pr2 + 0, 0:64],
                                        pst2[:, j * P:j * P + 64])
                                    nc.vector.tensor_copy(
                                        vq_v[:, kt, 2 * pr2 + 1, 0:64],
                                        pst2[:, j * P + 64:(j + 1) * P])

                        if KDBG and q4 == 0:
                            nc.gpsimd.dma_start(dbg_kt[:, :], kts[0])
                            nc.gpsimd.dma_start(dbg_vq[:, :, :], vq)

                        # attention for the quad's two pairs
                        for pr2 in range(2):
                            m = q4 * 2 + pr2
                            ktp = kts[pr2]
                            opsA = pso.tile([65, 1024], F32, tag="opsum")
                            opsB = pso.tile([65, 1024], F32, tag="opsum")
                            for kt in range(KT):
                                ksl = slice(kt * P, (kt + 1) * P)
                                for hh, ops in ((0, opsA), (1, opsB)):
                                    rows = slice(hh * 64, hh * 64 + 64)
                                    sc = ps.tile([P, 1024], F32, tag="ps")
                                    for c in range(QC):
                                        qsl = slice(c * 512, (c + 1) * 512)
                                        nc.tensor.matmul(
                                            sc[:, c * 512:(c + 1) * 512],
                                            ktp[rows, ksl],
                                            qt_all[rows, m, qsl],
                                            start=True, stop=True)
                                    pt = ppool.tile([P, 1024], BF16,
                                                    tag="pT")
                                    _ei = nc.scalar.activation(
                                        pt[:, 0:TQ], sc[:, 0:TQ], AF.Exp)
                                    act_exp_insts.append(_ei)
                                    add_dep_helper(
                                        _ei.ins, dummy_exp.ins, sync=True,
                                        reason="act table: exp after switch")
                                    nc.vector.tensor_tensor(
                                        pt[:, 0:TQ], pt[:, 0:TQ],
                                        mask_sb[:, kt], A.mult)
                                    h4 = 2 * pr2 + hh
                                    vcols = slice(h4 * 65, h4 * 65 + 65)
                                    for c in range(QC):
                                        nc.tensor.matmul(
                                            ops[:, c * 512:(c + 1) * 512],
                                            vq[:, kt, vcols],
                                            pt[:, c * 512:(c + 1) * 512],
                                            start=(kt == 0),
                                            stop=(kt == KT - 1))
                            # normalize: hidden = O / sum (sum at row 64)
                            for hh, ops in ((0, opsA), (1, opsB)):
                                ssb = tmp.tile([65, TQ], F32, tag="ssb")
                                nc.vector.reciprocal(ssb[64:65],
                                                     ops[64:65, 0:TQ])
                                rb = tmp.tile([64, TQ], F32, tag="t4")
                                bcast_rows(rb, ssb[64:65, :], 64, TQ)
                                if hh == 0:
                                    nc.vector.tensor_tensor(
                                        hidden[0:64, m], ops[0:64, 0:TQ],
                                        rb, A.mult)
                                else:
                                    hb = tmp.tile([64, TQ], BF16, tag="hb")
                                    nc.vector.tensor_tensor(
                                        hb, ops[0:64, 0:TQ], rb, A.mult)
                                    nc.gpsimd.dma_start(hidden[64:128, m],
                                                        hb)

        # ---------- Wo + residual, LN2, FFN ----------
        # (ln_stats computes per-token mean/rstd over E via ones-matmul)
        def ln_stats(src_sb, ntok, foldrow, a_bcast):
            nchunk = ntok // 512
            w = 512
            for c in range(nchunk):
                sl = slice(c * w, (c + 1) * w)
                pst = ps.tile([P, 1024], F32, tag="ps")
                psum_s = pst[0:1, 0:w]
                psum_q = pst[0:1, 512:512 + w]
                for s in range(ES):
                    nc.tensor.matmul(psum_s, ones_bf, src_sb[:, s, sl],
                                     start=(s == 0), stop=(s == ES - 1))
                for s in range(ES):
                    sq_s = tmp.tile([P, w], BF16, tag="sq_s")
                    nc.vector.tensor_mul(sq_s, src_sb[:, s, sl],
                                         src_sb[:, s, sl])
                    nc.tensor.matmul(psum_q, ones_bf, sq_s,
                                     start=(s == 0), stop=(s == ES - 1))
                mu = tmp.tile([1, w], F32, tag="mu")
                nc.vector.tensor_scalar_mul(mu, psum_s, 1.0 / E)
                m2 = tmp.tile([1, w], F32, tag="m2")
                nc.vector.tensor_scalar_mul(m2, psum_q, 1.0 / E)
                var = tmp.tile([1, w], F32, tag="var")
                nc.vector.tensor_mul(var, mu, mu)
                nc.vector.tensor_sub(var, m2, var)
                nc.vector.tensor_scalar_add(var, var, EPS)
                # r = rsqrt(var) via reciprocal seed + 3 Newton steps
                w_ = tmp.tile([1, w], F32, tag="wrec")
                nc.vector.reciprocal(w_, var)
                r_ = tmp.tile([1, w], F32, tag="rr")
                nc.vector.tensor_scalar(r_, w_, 0.5, 0.5, A.mult, A.add)
                t_ = tmp.tile([1, w], F32, tag="tt")
                for _ in range(3):
                    nc.vector.tensor_mul(t_, r_, r_)
                    nc.vector.tensor_mul(t_, t_, var)
                    nc.vector.tensor_scalar(t_, t_, -0.5, 1.5,
                                            A.mult, A.add)
                    nc.vector.tensor_mul(r_, r_, t_)
                irow = tmp.tile([1, w], F32, tag="irow")
                nc.vector.tensor_mul(irow, var, r_)
                nc.vector.tensor_copy(a_bcast[0:1, sl], r_)
                nc.vector.tensor_scalar_mul(foldrow[0:1, sl], mu, -1.0)
                nc.gpsimd.dma_start(foldrow[1:2, sl], irow)
            bcast_rows(a_bcast[1:P, :], a_bcast[0:1, :], P - 1, ntok)

        if KDBG:
            nc.gpsimd.dma_start(dbg_hid[:, :, :], hidden)

        with tc.tile_pool(name="post", bufs=1) as post:
            out1 = post.tile([P, ES, TQ], F32, tag="out1")
            out1bf = post.tile([P, ES, TQ], BF16, tag="out1bf")
            for et in range(ES):
                wo_et = wstream.tile([P, ES, P], BF16, tag="w")
                nc.sync.dma_start(wo_et, wo3[:, :, et * P:(et + 1) * P])
                pst = ps.tile([P, 1024], F32, tag="ps")
                for c in range(QC):
                    psl = pst[:, c * 512:(c + 1) * 512]
                    qsl = slice(c * 512, (c + 1) * 512)
                    for s in range(ES):
                        nc.tensor.matmul(psl, wo_et[:, s], hidden[:, s, qsl],
                                         start=(s == 0), stop=(s == ES - 1))
                xr = tmp.tile([P, TQ], BF16, tag="xr")
                nc.sync.dma_start(xr, xq3[:, et])
                nc.vector.tensor_add(out1[:, et], pst[:, 0:TQ], xr)
                nc.vector.tensor_copy(out1bf[:, et], out1[:, et])

            foldrow2 = post.tile([P, TQ], BF16, tag="foldrow2")
            nc.vector.memset(foldrow2, 0.0)
            a2 = post.tile([P, TQ], BF16, tag="a2")
            ln_stats(out1bf, TQ, foldrow2, a2)

            scr_out2 = perm.tile([1, 8], F32, tag="scr_out2")
            dummy_gelu = nc.scalar.activation(scr_out2, scr_in, AF.Gelu)
            for ei in act_exp_insts:
                add_dep_helper(dummy_gelu.ins, ei.ins, sync=True,
                               reason="act table: gelu after all exps")

            # w1 fold stationary: row0 = colsum(W1'), rows 1.. = 0
            w1f_sb = post.tile([P, FF], BF16, tag="w1f")
            nc.vector.memset(w1f_sb, 0.0)
            nc.gpsimd.dma_start(w1f_sb[0:1, :], fo_d[2:3, :])

            h3 = post.tile([P, FS, TQ], BF16, tag="h3")
            for ft in range(FS):
                w1_ft = wstream.tile([P, ES, P], BF16, tag="w")
                nc.sync.dma_start(w1_ft, w13[:, :, ft * P:(ft + 1) * P])
                pst = ps.tile([P, 1024], F32, tag="ps")
                for c in range(QC):
                    psl = pst[:, c * 512:(c + 1) * 512]
                    qsl = slice(c * 512, (c + 1) * 512)
                    for s in range(ES):
                        nc.tensor.matmul(psl, w1_ft[:, s], out1bf[:, s, qsl],
                                         start=(s == 0), stop=False)
                    nc.tensor.matmul(psl, w1f_sb[:, ft * P:(ft + 1) * P],
                                     foldrow2[:, qsl], start=False, stop=True)
                mid = tmp.tile([P, TQ], F32, tag="t4")
                nc.vector.tensor_tensor(mid, pst[:, 0:TQ], a2, A.mult)
                gi = nc.scalar.activation(h3[:, ft], mid, AF.Gelu,
                                          bias=b1f_sb[:, ft:ft + 1])
                add_dep_helper(gi.ins, dummy_gelu.ins, sync=True,
                               reason="act table: gelu after switch")

            for et in range(ES):
                pst = ps.tile([P, 1024], F32, tag="ps")
                for part in range(FS // ES):
                    w2_et = wstream.tile([P, ES, P], BF16, tag="w")
                    nc.sync.dma_start(
                        w2_et,
                        w23[:, part * ES:(part + 1) * ES,
                            et * P:(et + 1) * P])
                    for c in range(QC):
                        psl = pst[:, c * 512:(c + 1) * 512]
                        qsl = slice(c * 512, (c + 1) * 512)
                        for s8 in range(ES):
                            s = part * ES + s8
                            nc.tensor.matmul(psl, w2_et[:, s8],
                                             h3[:, s, qsl],
                                             start=(s == 0),
                                             stop=(s == FS - 1))
                ot = tmp.tile([P, TQ], BF16, tag="ot")
                nc.vector.scalar_tensor_tensor(
                    ot, pst[:, 0:TQ], b2f_sb[:, et:et + 1], out1[:, et],
                    A.add, A.add)
                nc.sync.dma_start(out3[:, et], ot)

    nc.compile()
    return nc


# ---------------------------------------------------------------------------
# Host side
# ---------------------------------------------------------------------------

def prep_inputs(dims, x, ln1_g, ln1_b, Wq, Wk, Wv, Wo, ln2_g, ln2_b,
                W1, b1, W2, b2):
    """Build per-core in_maps (list of dicts keyed by dram tensor names)."""
    B, C, E, H, D, FF = (dims["B"], dims["C"], dims["E"], dims["H"],
                         dims["D"], dims["FF"])
    EPS = dims["EPS"]
    TQ = B * C // N_CORES
    KT = C // P
    FS = FF // P
    ES = E // P
    bf = ml_dtypes.bfloat16
    f32 = np.float32

    x = np.asarray(x, f32)
    g1 = np.asarray(ln1_g, f32)
    b1_ = np.asarray(ln1_b, f32)
    sc = 1.0 / math.sqrt(D)
    wq = g1[:, None] * np.asarray(Wq, f32) * sc
    wk = g1[:, None] * np.asarray(Wk, f32)
    wv = g1[:, None] * np.asarray(Wv, f32)
    w1 = np.asarray(ln2_g, f32)[:, None] * np.asarray(W1, f32)
    b1f = np.asarray(b1, f32) + np.asarray(ln2_b, f32) @ np.asarray(W1, f32)

    # blobA is gathered in two collectives (qkv rows, then wo|w2 rows),
    # so each per-core shard must hold its qkv slice first.
    qkv_rows = np.concatenate([wq, wk, wv], axis=0).astype(bf)   # [3E, E]
    rest_rows = np.concatenate(
        [np.asarray(Wo, f32), np.asarray(W2, f32)], axis=0).astype(bf)
    blobB = w1.astype(bf)                      # [E, FF]
    QSH = qkv_rows.shape[0] // N_CORES         # 384
    RSH = rest_rows.shape[0] // N_CORES        # 640
    BSH = blobB.shape[0] // N_CORES

    folds = np.zeros((4, FF), f32)
    folds[0, 0:E] = wq.sum(0)
    folds[0, E:2 * E] = wk.sum(0)
    folds[0, 2 * E:3 * E] = wv.sum(0)
    folds[1, 0:E] = b1_ @ wq
    folds[1, E:2 * E] = b1_ @ wk
    folds[1, 2 * E:3 * E] = b1_ @ wv
    folds[2, :] = w1.sum(0)

    mu = x.mean(-1)                            # [B, C]
    var = x.var(-1)
    sd = np.sqrt(var + EPS)

    kidx = (np.arange(KT)[None, :] * P
            + np.arange(P)[:, None]).astype(f32)

    shared = {
        "folds": folds,
        "kidx": kidx,
        "ident": np.eye(P, dtype=f32).astype(bf),
        "b1f": np.ascontiguousarray(b1f.reshape(FS, P).T),
        "b2f": np.ascontiguousarray(
            np.asarray(b2, f32).reshape(ES, P).T),
    }

    nhalf = C // TQ  # query shards per batch (2)
    in_maps = []
    for c in range(N_CORES):
        b = c // nhalf
        off = (c % nhalf) * TQ
        stats = np.zeros((4, TQ), f32)
        stats[0] = -mu[b, off:off + TQ]
        stats[1] = sd[b, off:off + TQ]
        stats[2] = 1.0 / sd[b, off:off + TQ]
        m = {
            "xq": np.ascontiguousarray(x[b, off:off + TQ].T).astype(bf),
            "stats": stats,
            "qoff": (off + np.arange(TQ, dtype=f32))[None, :],
            "wa": np.concatenate(
                [qkv_rows[c * QSH:(c + 1) * QSH],
                 rest_rows[c * RSH:(c + 1) * RSH]], axis=0),
            "wb": np.ascontiguousarray(blobB[c * BSH:(c + 1) * BSH]),
        }
        m.update(shared)
        in_maps.append(m)
    return in_maps


def assemble_output(dims, results):
    B, C, E = dims["B"], dims["C"], dims["E"]
    TQ = B * C // N_CORES
    nhalf = C // TQ
    out = np.empty((B, C, E), np.float32)
    for c in range(N_CORES):
        b = c // nhalf
        off = (c % nhalf) * TQ
        out[b, off:off + TQ] = np.asarray(
            results[c]["outT"], dtype=np.float32).T
    return out


def kernel(**inputs):
    dims = DIMS
    nc = build_program(dims)
    in_maps = prep_inputs(dims, **{k: np.asarray(v) for k, v in
                                   inputs.items()})
    res = run_bass_kernel_spmd(nc, in_maps, list(range(N_CORES)))
    return assemble_output(dims, res.results)


if __name__ == "__main__":
    nc = build_program(DIMS)
    print("build ok")
